# revision 8
# baseline (speedup 1.0000x reference)
"""Trainium2 Bass kernel for nn_Cov_EBFLayer (v2: triangular s^2 feature map).

Math: out[b,o] = exp(-quad[o,b]),
  quad[o,b] = diff^T P_o diff,  diff = c_o - x_b,  P_o = B_o B_o^T
            = sum_{d<f} P[d,f] (x_d+x_f)^2            <- s^2 features (2016)
            + sum_d (2 P[d,d] - rowsum_d) x_d^2       <- diag features (64)
            - 2 v_o . x + q3_o                        <- linear (64) + bias
  The (x_d+x_f)^2 expansion absorbs the symmetric cross terms; its spurious
  x^2 terms are folded into the diagonal coefficients (host, ~2M MACs).

Kernel (per core, batch-sharded 8 x 1024):
  - s-chunks: 16 chunks of 128 (d<f) pairs. s = indc-matmul (two-hot, K=64,
    row-tiled by chunk parity), g = Square(s) on ACT/DVE straight from PSUM.
  - chunk 0: [x^2 (DVE square); x] with host-computed coefficients.
  - Gram on PE: 128 pair-Grams row-tiled 2x (even pairs K=0:64, odd 64:128),
    PSUM -> SBUF strided copies (ACT+DVE) into p_sb[f, (f2, o')],
    o' = (o%2)*128 + o//2 (parity-major device order).
  - W: p_sb -> DRAM (2MB) -> triangle-row gather DMAs -> W[(slot), (c, o')].
  - mains: per o-half (serial, 2 PSUM banks): 17 accumulating K<=128 matmuls
    per b-tile; epilogue EXP(scale=-1, bias=-q3[o']) -> fp16 out.
"""

import sys
from contextlib import ExitStack

import numpy as np

sys.path.insert(0, "/opt/trn_rl_repo")

import concourse.bass as bass  # noqa: E402
import concourse.tile as tile  # noqa: E402
from concourse import bacc, mybir  # noqa: E402
from concourse import bass_utils  # noqa: E402
from concourse._compat import with_exitstack  # noqa: E402

B, D, O, NCORES = 8192, 64, 256, 8
BSH = B // NCORES  # 1024 per-core batch shard
BT = 512
NBT = BSH // BT  # 2
NSC = 16  # s^2 chunks (2016 strict-upper pairs -> 15.75 chunks)
NCHUNK = NSC + 1  # + diag/linear chunk 0
F32 = mybir.dt.float32
F16 = mybir.dt.float16

# slot maps (compile-time constants)
_TRIU_D, _TRIU_F = np.triu_indices(D, k=1)  # 2016 pairs, row-major


def _row_segments():
    """Per off-diag row d: list of (chunk, part0, f0, seglen) covering its slots."""
    segs = []
    s = 0
    for d in range(D - 1):
        length = D - 1 - d
        f0 = d + 1
        while length > 0:
            c = 1 + s // 128
            p0 = s % 128
            seg = min(length, 128 - p0)
            segs.append((d, c, p0, f0, seg))
            s += seg
            f0 += seg
            length -= seg
    return segs


@with_exitstack
def _kernel(ctx: ExitStack, tc, outT, xT, betasT, indc, linW, negq3):
    nc = tc.nc

    cpool = ctx.enter_context(tc.tile_pool(name="const", bufs=1))
    dpool = ctx.enter_context(tc.tile_pool(name="dram", bufs=1, space="DRAM"))
    ge_pool = ctx.enter_context(tc.tile_pool(name="psum_ge", bufs=2, space="PSUM"))
    go_pool = ctx.enter_context(tc.tile_pool(name="psum_go", bufs=2, space="PSUM"))
    s_pool = ctx.enter_context(tc.tile_pool(name="psum_s", bufs=2, space="PSUM"))
    q_pool = ctx.enter_context(tc.tile_pool(name="psum_q", bufs=1, space="PSUM"))
    opool = ctx.enter_context(tc.tile_pool(name="outs", bufs=4))

    # ---- resident inputs (small ones first so PE work can start early) ----
    xb = cpool.tile([128, BSH], F16)  # [xT; xT]
    nc.sync.dma_start(xb[0:D, :], xT[:])
    nc.sync.dma_start(xb[D:128, :], xT[:])
    sb_indc = cpool.tile([128, 8 * 128], F16)
    nc.sync.dma_start(sb_indc[:], indc[:])
    w = cpool.tile([128, NCHUNK * 256], F16)  # [slot, (chunk, o')]
    nc.sync.dma_start(w[:, 0:256], linW[:])  # chunk 0 from host
    sb_nq3 = cpool.tile([128, 2], F32)
    nc.sync.dma_start(sb_nq3[:], negq3[:])
    sb_betasT = cpool.tile([128, 64 * 128], F16)  # row r: pairs tt%2==r
    for k in range(4):
        nc.sync.dma_start(
            sb_betasT[:, k * 2048 : (k + 1) * 2048],
            betasT[:, k * 2048 : (k + 1) * 2048],
        )

    # ---- g chunks ----
    g = [
        cpool.tile([128, BSH], F16, name=f"g{c}", uniquify=False)
        for c in range(NCHUNK)
    ]
    # chunk 0: [x^2 ; x]
    nc.sync.dma_start(g[0][D:128, :], xT[:])
    nc.vector.tensor_mul(g[0][0:D, :], xb[0:D, :], xb[0:D, :])

    # ---- s-chunks: indicator matmuls + squares (also the PE warm-up) ----
    for c in range(1, NSC + 1):
        b = (c - 1) // 2
        r0 = 0 if c % 2 == 1 else 64
        for bt in range(NBT):
            ps = s_pool.tile([128, BT], F32, name=f"s_{c}_{bt}", tag="s")
            nc.tensor.matmul(
                ps[:],
                sb_indc[r0 : r0 + 64, b * 128 : (b + 1) * 128],
                xb[r0 : r0 + 64, bt * BT : (bt + 1) * BT],
                start=True,
                stop=True,
            )
            eng = nc.scalar if (c + bt) % 2 == 0 else nc.vector
            if (c + bt) % 2 == 0:
                eng.activation(
                    g[c][:, bt * BT : (bt + 1) * BT],
                    ps[:],
                    mybir.ActivationFunctionType.Square,
                )
            else:
                # DVE: copy+mul (no single-op square from PSUM on DVE)
                tmp = opool.tile([128, BT], F16, name=f"st_{c}_{bt}", tag="st")
                eng.tensor_copy(tmp[:], ps[:])
                eng.tensor_mul(
                    g[c][:, bt * BT : (bt + 1) * BT], tmp[:], tmp[:]
                )

    # ---- Gram: row-tiled pair matmuls + strided copies to p_sb ----
    p_sb = cpool.tile([D, D * 256], F16)  # [f, (f2, o')]
    for gidx in range(16):  # m-groups of 4
        for r in range(2):  # tile-row: even/odd pairs
            pool = ge_pool if r == 0 else go_pool
            pp = pool.tile([128, 512], F32, name=f"pp_{gidx}_{r}", tag=f"pp{r}")
            for q in range(4):
                m = gidx * 4 + q
                blk = sb_betasT[r * 64 : r * 64 + 64, m * 128 : (m + 1) * 128]
                nc.tensor.matmul(
                    pp[:, q * 128 : (q + 1) * 128], blk, blk, start=True, stop=True
                )
            # copies: src partitions (j, f1), free (q, f2 of j2==j);
            # dst p_sb[f1, f2*256 + j*128 + 8*gidx + 2*q + r]
            for j in range(2):
                src = pp[j * 64 : (j + 1) * 64, :].rearrange(
                    "p (q j2 f) -> p q j2 f", q=4, j2=2
                )[:, :, j, :]
                # dst: iterate (q:4 at o'-stride 2, f2:64); o' = j*128+8*gidx+2q+r
                dstv = p_sb[:].rearrange(
                    "p (f j gi q r) -> p j gi r q f", j=2, gi=16, q=4, r=2
                )
                dst = dstv[:, j, gidx, r, :, :]
                if (gidx + j) % 2 == 0:
                    nc.scalar.activation(
                        dst, src, mybir.ActivationFunctionType.Copy
                    )
                else:
                    nc.vector.tensor_copy(dst, src)

    # ---- W path: p_sb -> DRAM -> triangle gather ----
    p_dram = dpool.tile([D, D * 256], F16)
    for k in range(4):
        nc.sync.dma_start(
            p_dram[k * 16 : (k + 1) * 16, :], p_sb[k * 16 : (k + 1) * 16, :]
        )
    p_dram_v = p_dram[:].rearrange("p (f o) -> p f o", o=256)
    for d, c, p0, f0, seg in _row_segments():
        nc.sync.dma_start(
            w[p0 : p0 + seg, c * 256 : (c + 1) * 256],
            p_dram_v[d, f0 : f0 + seg, :],
        )

    # ---- mains: o-half serial; 2 PSUM banks (one per b-tile) ----
    for oh in range(2):
        pq = [
            q_pool.tile([128, BT], F32, name=f"pq_{oh}_{bt}", tag=f"pq{bt}")
            for bt in range(NBT)
        ]
        for c in range(NCHUNK):
            kc = 96 if c == NSC else 128
            for bt in range(NBT):
                nc.tensor.matmul(
                    pq[bt][:],
                    w[0:kc, c * 256 + oh * 128 : c * 256 + oh * 128 + 128],
                    g[c][0:kc, bt * BT : (bt + 1) * BT],
                    start=(c == 0),
                    stop=(c == NCHUNK - 1),
                )
        for bt in range(NBT):
            ob = opool.tile([128, BT], F16, name=f"ob_{oh}_{bt}", tag="ob")
            nc.scalar.activation(
                ob[:],
                pq[bt][:],
                mybir.ActivationFunctionType.Exp,
                scale=-1.0,
                bias=sb_nq3[:, oh : oh + 1],
            )
            nc.sync.dma_start(
                outT[oh * 128 : (oh + 1) * 128, bt * BT : (bt + 1) * BT], ob[:]
            )


_CACHE = {}


def _build():
    if "nc" in _CACHE:
        return _CACHE["nc"], _CACHE["aps"]
    nc = bacc.Bacc(
        "TRN2", target_bir_lowering=False, debug=False, num_devices=NCORES
    )
    xT = nc.dram_tensor("xT", [D, BSH], F16, kind="ExternalInput").ap()
    betasT = nc.dram_tensor("betasT", [128, 64 * 128], F16, kind="ExternalInput").ap()
    indc = nc.dram_tensor("indc", [128, 8 * 128], F16, kind="ExternalInput").ap()
    linW = nc.dram_tensor("linW", [128, 256], F16, kind="ExternalInput").ap()
    negq3 = nc.dram_tensor("negq3", [128, 2], F32, kind="ExternalInput").ap()
    outT = nc.dram_tensor("outT", [O, BSH], F16, kind="ExternalOutput").ap()
    with tile.TileContext(nc) as tc:
        _kernel(tc, outT, xT, betasT, indc, linW, negq3)
    nc.compile()
    _CACHE["nc"] = nc
    _CACHE["aps"] = (xT, betasT, indc, linW, negq3, outT)
    return nc, _CACHE["aps"]


def _operm():
    o = np.arange(O)
    return (o % 2) * 128 + o // 2


def _host_prep(x, centers, betas):
    x = np.asarray(x, np.float32)
    betas = np.asarray(betas, np.float32)
    c = np.asarray(centers, np.float32).reshape(O, D)
    operm = _operm()

    # betasT stacked for row-tiled Gram: row-block r holds pairs tt with
    # tt%2==r; block m covers pair tt=2m+r -> o = (4m+2r, 4m+2r+1)
    bt_all = betas.transpose(2, 0, 1)  # [e, o, f]
    ev = np.stack([bt_all[:, 4 * m : 4 * m + 2, :] for m in range(64)], axis=1)
    od = np.stack([bt_all[:, 4 * m + 2 : 4 * m + 4, :] for m in range(64)], axis=1)
    betasT = np.concatenate(
        [ev.reshape(D, 64 * 128), od.reshape(D, 64 * 128)], axis=0
    ).astype(np.float16)

    # indc: two-hot columns for s-chunks; chunk c odd -> rows 0:64 of block
    # (c-1)//2, c even -> rows 64:128
    indc = np.zeros((128, 8 * 128), np.float32)
    for cc in range(1, NSC + 1):
        b = (cc - 1) // 2
        r0 = 0 if cc % 2 == 1 else 64
        lo = (cc - 1) * 128
        hi = min(lo + 128, 2016)
        for p in range(hi - lo):
            s = lo + p
            indc[r0 + _TRIU_D[s], b * 128 + p] = 1.0
            indc[r0 + _TRIU_F[s], b * 128 + p] = 1.0
    indc = indc.astype(np.float16)

    # host linear prep (~5M MACs)
    s1 = betas.sum(axis=1)  # [O, e]
    rowsum = np.einsum("ode,oe->od", betas, s1)  # [O, d]
    pdd = (betas ** 2).sum(axis=2)  # [O, d]
    diagW = 2.0 * pdd - rowsum  # [O, d]
    wvec = np.einsum("ofe,of->oe", betas, c)
    v = np.einsum("ode,oe->od", betas, wvec)
    q3 = (wvec ** 2).sum(axis=1)

    linW = np.zeros((128, 256), np.float32)
    linW[0:D, operm] = diagW.T
    linW[D:128, operm] = (-2.0 * v).T
    linW = linW.astype(np.float16)

    negq3 = np.zeros((128, 2), np.float32)
    for oh in range(2):
        inv = np.empty(O, np.int64)
        inv[operm] = np.arange(O)
        negq3[:, oh] = -q3[inv[oh * 128 : (oh + 1) * 128]]

    xT_shards = [
        np.ascontiguousarray(x[i * BSH : (i + 1) * BSH].T).astype(np.float16)
        for i in range(NCORES)
    ]
    return xT_shards, betasT, indc, linW, negq3


def _run(x, centers, betas, trace=False):
    nc, (xT, betasT_ap, indc_ap, linW_ap, negq3_ap, outT) = _build()
    xT_shards, betasT, indc, linW, negq3 = _host_prep(x, centers, betas)
    in_maps = [
        {
            xT.name: xT_shards[i],
            betasT_ap.name: betasT,
            indc_ap.name: indc,
            linW_ap.name: linW,
            negq3_ap.name: negq3,
        }
        for i in range(NCORES)
    ]
    res = bass_utils.run_bass_kernel_spmd(
        nc, in_maps, core_ids=list(range(NCORES)), trace=trace
    )
    operm = _operm()
    out = np.concatenate(
        [
            np.asarray(res.results[i][outT.name])[operm, :].T.astype(np.float32)
            for i in range(NCORES)
        ],
        axis=0,
    )
    return out, res


def kernel(x, centers, betas):
    out, _ = _run(x, centers, betas, trace=False)
    return out


# revision 11
# speedup vs baseline: 1.1751x; 1.1751x over previous
"""Trainium2 Bass kernel for nn_Cov_EBFLayer (v2: triangular s^2 feature map).

Math: out[b,o] = exp(-quad[o,b]),
  quad[o,b] = diff^T P_o diff,  diff = c_o - x_b,  P_o = B_o B_o^T
            = sum_{d<f} P[d,f] (x_d+x_f)^2            <- s^2 features (2016)
            + sum_d (2 P[d,d] - rowsum_d) x_d^2       <- diag features (64)
            - 2 v_o . x + q3_o                        <- linear (64) + bias
  The (x_d+x_f)^2 expansion absorbs the symmetric cross terms; its spurious
  x^2 terms are folded into the diagonal coefficients (host, ~2M MACs).

Kernel (per core, batch-sharded 8 x 1024):
  - s-chunks: 16 chunks of 128 (d<f) pairs. s = indc-matmul (two-hot, K=64,
    row-tiled by chunk parity), g = Square(s) on ACT/DVE straight from PSUM.
  - chunk 0: [x^2 (DVE square); x] with host-computed coefficients.
  - Gram on PE: 128 pair-Grams row-tiled 2x (even pairs K=0:64, odd 64:128),
    PSUM -> SBUF strided copies (ACT+DVE) into p_sb[f, (f2, o')],
    o' = (o%2)*128 + o//2 (parity-major device order).
  - W: p_sb -> DRAM (2MB) -> triangle-row gather DMAs -> W[(slot), (c, o')].
  - mains: per o-half (serial, 2 PSUM banks): 17 accumulating K<=128 matmuls
    per b-tile; epilogue EXP(scale=-1, bias=-q3[o']) -> fp16 out.
"""

import sys
from contextlib import ExitStack

import numpy as np

sys.path.insert(0, "/opt/trn_rl_repo")

import concourse.bass as bass  # noqa: E402
import concourse.tile as tile  # noqa: E402
from concourse import bacc, mybir  # noqa: E402
from concourse import bass_utils  # noqa: E402
from concourse._compat import with_exitstack  # noqa: E402

B, D, O, NCORES = 8192, 64, 256, 8
BSH = B // NCORES  # 1024 per-core batch shard
BT = 512
NBT = BSH // BT  # 2
NSC = 16  # s^2 chunks (2016 strict-upper pairs -> 15.75 chunks)
NCHUNK = NSC + 1  # + diag/linear chunk 0
F32 = mybir.dt.float32
F16 = mybir.dt.float16

# slot maps (compile-time constants)
_TRIU_D, _TRIU_F = np.triu_indices(D, k=1)  # 2016 pairs, row-major


def _row_segments():
    """Per off-diag row d: list of (chunk, part0, f0, seglen) covering its slots."""
    segs = []
    s = 0
    for d in range(D - 1):
        length = D - 1 - d
        f0 = d + 1
        while length > 0:
            c = 1 + s // 128
            p0 = s % 128
            seg = min(length, 128 - p0)
            segs.append((d, c, p0, f0, seg))
            s += seg
            f0 += seg
            length -= seg
    return segs


@with_exitstack
def _kernel(ctx: ExitStack, tc, outT, xT, betasT, indc, linW, negq3):
    nc = tc.nc

    cpool = ctx.enter_context(tc.tile_pool(name="const", bufs=1))
    dpool = ctx.enter_context(tc.tile_pool(name="dram", bufs=1, space="DRAM"))
    ge_pool = ctx.enter_context(tc.tile_pool(name="psum_ge", bufs=2, space="PSUM"))
    go_pool = ctx.enter_context(tc.tile_pool(name="psum_go", bufs=2, space="PSUM"))
    s_pool = ctx.enter_context(tc.tile_pool(name="psum_s", bufs=2, space="PSUM"))
    q_pool = ctx.enter_context(tc.tile_pool(name="psum_q", bufs=1, space="PSUM"))
    opool = ctx.enter_context(tc.tile_pool(name="outs", bufs=4))

    # ---- resident inputs (small ones first so PE work can start early) ----
    xb = cpool.tile([128, BSH], F16)  # [xT; xT]
    nc.sync.dma_start(xb[0:D, :], xT[:])
    nc.sync.dma_start(xb[D:128, :], xT[:])
    sb_indc = cpool.tile([128, 8 * 128], F16)
    nc.sync.dma_start(sb_indc[:], indc[:])
    w = cpool.tile([128, NCHUNK * 256], F16)  # [slot, (chunk, o')]
    nc.sync.dma_start(w[:, 0:256], linW[:])  # chunk 0 from host
    sb_nq3 = cpool.tile([128, 2], F32)
    nc.sync.dma_start(sb_nq3[:], negq3[:])
    sb_betasT = cpool.tile([128, 64 * 128], F16)  # row r: pairs tt%2==r
    for k in range(4):
        nc.sync.dma_start(
            sb_betasT[:, k * 2048 : (k + 1) * 2048],
            betasT[:, k * 2048 : (k + 1) * 2048],
        )

    # ---- g chunks ----
    g = [
        cpool.tile([128, BSH], F16, name=f"g{c}", uniquify=False)
        for c in range(NCHUNK)
    ]
    # chunk 0: [x^2 ; x]
    nc.sync.dma_start(g[0][D:128, :], xT[:])
    nc.vector.tensor_mul(g[0][0:D, :], xb[0:D, :], xb[0:D, :])

    # ---- s-chunks: indicator matmuls + squares (also the PE warm-up) ----
    for c in range(1, NSC + 1):
        b = (c - 1) // 2
        r0 = 0 if c % 2 == 1 else 64
        for bt in range(NBT):
            ps = s_pool.tile([128, BT], F32, name=f"s_{c}_{bt}", tag="s")
            nc.tensor.matmul(
                ps[:],
                sb_indc[r0 : r0 + 64, b * 128 : (b + 1) * 128],
                xb[r0 : r0 + 64, bt * BT : (bt + 1) * BT],
                start=True,
                stop=True,
            )
            nc.scalar.activation(
                g[c][:, bt * BT : (bt + 1) * BT],
                ps[:],
                mybir.ActivationFunctionType.Square,
            )

    # ---- Gram: row-tiled pair matmuls + strided copies to p_sb ----
    p_sb = cpool.tile([D, D * 256], F16)  # [f, (f2, o')]
    for gidx in range(16):  # m-groups of 4
        for r in range(2):  # tile-row: even/odd pairs
            pool = ge_pool if r == 0 else go_pool
            pp = pool.tile([128, 512], F32, name=f"pp_{gidx}_{r}", tag=f"pp{r}")
            for q in range(4):
                m = gidx * 4 + q
                blk = sb_betasT[r * 64 : r * 64 + 64, m * 128 : (m + 1) * 128]
                nc.tensor.matmul(
                    pp[:, q * 128 : (q + 1) * 128], blk, blk, start=True, stop=True
                )
            # copies: src partitions (j, f1), free (f2 of j2==j, q);
            # dst p_sb[f1, f2*256 + (j*128 + r*64 + 4*gidx + q)] -- 4-contig o'
            for j in range(2):
                src = pp[j * 64 : (j + 1) * 64, :].rearrange(
                    "p (q j2 f) -> p j2 f q", q=4, j2=2
                )[:, j, :, :]
                dstv = p_sb[:].rearrange(
                    "p (f j r gi q) -> p j r gi f q", j=2, r=2, gi=16, q=4
                )
                dst = dstv[:, j, r, gidx, :, :]
                cidx = gidx * 4 + r * 2 + j
                if cidx % 8 == 0:  # 8 of 64 copies on ACT, rest DVE
                    nc.scalar.activation(
                        dst, src, mybir.ActivationFunctionType.Copy
                    )
                else:
                    nc.vector.tensor_copy(dst, src)

    # ---- W path: p_sb -> DRAM -> triangle gather ----
    p_dram = dpool.tile([D, D * 256], F16)
    for k in range(4):
        nc.sync.dma_start(
            p_dram[k * 16 : (k + 1) * 16, :], p_sb[k * 16 : (k + 1) * 16, :]
        )
    p_dram_v = p_dram[:].rearrange("p (f o) -> p f o", o=256)
    for d, c, p0, f0, seg in _row_segments():
        nc.sync.dma_start(
            w[p0 : p0 + seg, c * 256 : (c + 1) * 256],
            p_dram_v[d, f0 : f0 + seg, :],
        )

    # ---- mains: o-half serial; 2 PSUM banks (one per b-tile) ----
    for oh in range(2):
        pq = [
            q_pool.tile([128, BT], F32, name=f"pq_{oh}_{bt}", tag=f"pq{bt}")
            for bt in range(NBT)
        ]
        for c in range(NCHUNK):
            kc = 96 if c == NSC else 128
            for bt in range(NBT):
                nc.tensor.matmul(
                    pq[bt][:],
                    w[0:kc, c * 256 + oh * 128 : c * 256 + oh * 128 + 128],
                    g[c][0:kc, bt * BT : (bt + 1) * BT],
                    start=(c == 0),
                    stop=(c == NCHUNK - 1),
                )
        for bt in range(NBT):
            ob = opool.tile([128, BT], F16, name=f"ob_{oh}_{bt}", tag="ob")
            nc.scalar.activation(
                ob[:],
                pq[bt][:],
                mybir.ActivationFunctionType.Exp,
                scale=-1.0,
                bias=sb_nq3[:, oh : oh + 1],
            )
            nc.sync.dma_start(
                outT[oh * 128 : (oh + 1) * 128, bt * BT : (bt + 1) * BT], ob[:]
            )


_CACHE = {}


def _build():
    if "nc" in _CACHE:
        return _CACHE["nc"], _CACHE["aps"]
    nc = bacc.Bacc(
        "TRN2", target_bir_lowering=False, debug=False, num_devices=NCORES
    )
    xT = nc.dram_tensor("xT", [D, BSH], F16, kind="ExternalInput").ap()
    betasT = nc.dram_tensor("betasT", [128, 64 * 128], F16, kind="ExternalInput").ap()
    indc = nc.dram_tensor("indc", [128, 8 * 128], F16, kind="ExternalInput").ap()
    linW = nc.dram_tensor("linW", [128, 256], F16, kind="ExternalInput").ap()
    negq3 = nc.dram_tensor("negq3", [128, 2], F32, kind="ExternalInput").ap()
    outT = nc.dram_tensor("outT", [O, BSH], F16, kind="ExternalOutput").ap()
    with tile.TileContext(nc) as tc:
        _kernel(tc, outT, xT, betasT, indc, linW, negq3)
    nc.compile()
    _CACHE["nc"] = nc
    _CACHE["aps"] = (xT, betasT, indc, linW, negq3, outT)
    return nc, _CACHE["aps"]


def _operm():
    # o' = j*128 + r*64 + m,  j = o%2, tt = o//2, r = tt%2, m = tt//2
    o = np.arange(O)
    tt = o // 2
    return (o % 2) * 128 + (tt % 2) * 64 + tt // 2


def _host_prep(x, centers, betas):
    x = np.asarray(x, np.float32)
    betas = np.asarray(betas, np.float32)
    c = np.asarray(centers, np.float32).reshape(O, D)
    operm = _operm()

    # betasT stacked for row-tiled Gram: row-block r holds pairs tt with
    # tt%2==r; block m covers pair tt=2m+r -> o = (4m+2r, 4m+2r+1)
    bt_all = betas.transpose(2, 0, 1)  # [e, o, f]
    ev = np.stack([bt_all[:, 4 * m : 4 * m + 2, :] for m in range(64)], axis=1)
    od = np.stack([bt_all[:, 4 * m + 2 : 4 * m + 4, :] for m in range(64)], axis=1)
    betasT = np.concatenate(
        [ev.reshape(D, 64 * 128), od.reshape(D, 64 * 128)], axis=0
    ).astype(np.float16)

    # indc: two-hot columns for s-chunks; chunk c odd -> rows 0:64 of block
    # (c-1)//2, c even -> rows 64:128
    indc = np.zeros((128, 8 * 128), np.float32)
    for cc in range(1, NSC + 1):
        b = (cc - 1) // 2
        r0 = 0 if cc % 2 == 1 else 64
        lo = (cc - 1) * 128
        hi = min(lo + 128, 2016)
        for p in range(hi - lo):
            s = lo + p
            indc[r0 + _TRIU_D[s], b * 128 + p] = 1.0
            indc[r0 + _TRIU_F[s], b * 128 + p] = 1.0
    indc = indc.astype(np.float16)

    # host linear prep (~5M MACs)
    s1 = betas.sum(axis=1)  # [O, e]
    rowsum = np.einsum("ode,oe->od", betas, s1)  # [O, d]
    pdd = (betas ** 2).sum(axis=2)  # [O, d]
    diagW = 2.0 * pdd - rowsum  # [O, d]
    wvec = np.einsum("ofe,of->oe", betas, c)
    v = np.einsum("ode,oe->od", betas, wvec)
    q3 = (wvec ** 2).sum(axis=1)

    linW = np.zeros((128, 256), np.float32)
    linW[0:D, operm] = diagW.T
    linW[D:128, operm] = (-2.0 * v).T
    linW = linW.astype(np.float16)

    negq3 = np.zeros((128, 2), np.float32)
    for oh in range(2):
        inv = np.empty(O, np.int64)
        inv[operm] = np.arange(O)
        negq3[:, oh] = -q3[inv[oh * 128 : (oh + 1) * 128]]

    xT_shards = [
        np.ascontiguousarray(x[i * BSH : (i + 1) * BSH].T).astype(np.float16)
        for i in range(NCORES)
    ]
    return xT_shards, betasT, indc, linW, negq3


def _run(x, centers, betas, trace=False):
    nc, (xT, betasT_ap, indc_ap, linW_ap, negq3_ap, outT) = _build()
    xT_shards, betasT, indc, linW, negq3 = _host_prep(x, centers, betas)
    in_maps = [
        {
            xT.name: xT_shards[i],
            betasT_ap.name: betasT,
            indc_ap.name: indc,
            linW_ap.name: linW,
            negq3_ap.name: negq3,
        }
        for i in range(NCORES)
    ]
    res = bass_utils.run_bass_kernel_spmd(
        nc, in_maps, core_ids=list(range(NCORES)), trace=trace
    )
    operm = _operm()
    out = np.concatenate(
        [
            np.asarray(res.results[i][outT.name])[operm, :].T.astype(np.float32)
            for i in range(NCORES)
        ],
        axis=0,
    )
    return out, res


def kernel(x, centers, betas):
    out, _ = _run(x, centers, betas, trace=False)
    return out


# revision 13
# speedup vs baseline: 1.2917x; 1.0993x over previous
"""Trainium2 Bass kernel for nn_Cov_EBFLayer (v2: triangular s^2 feature map).

Math: out[b,o] = exp(-quad[o,b]),
  quad[o,b] = diff^T P_o diff,  diff = c_o - x_b,  P_o = B_o B_o^T
            = sum_{d<f} P[d,f] (x_d+x_f)^2            <- s^2 features (2016)
            + sum_d (2 P[d,d] - rowsum_d) x_d^2       <- diag features (64)
            - 2 v_o . x + q3_o                        <- linear (64) + bias
  The (x_d+x_f)^2 expansion absorbs the symmetric cross terms; its spurious
  x^2 terms are folded into the diagonal coefficients (host, ~2M MACs).

Kernel (per core, batch-sharded 8 x 1024):
  - s-chunks: 16 chunks of 128 (d<f) pairs. s = indc-matmul (two-hot, K=64,
    row-tiled by chunk parity), g = Square(s) on ACT/DVE straight from PSUM.
  - chunk 0: [x^2 (DVE square); x] with host-computed coefficients.
  - Gram on PE: 128 pair-Grams row-tiled 2x (even pairs K=0:64, odd 64:128),
    PSUM -> SBUF strided copies (ACT+DVE) into p_sb[f, (f2, o')],
    o' = (o%2)*128 + o//2 (parity-major device order).
  - W: p_sb -> DRAM (2MB) -> triangle-row gather DMAs -> W[(slot), (c, o')].
  - mains: per o-half (serial, 2 PSUM banks): 17 accumulating K<=128 matmuls
    per b-tile; epilogue EXP(scale=-1, bias=-q3[o']) -> fp16 out.
"""

import sys
from contextlib import ExitStack

import numpy as np

sys.path.insert(0, "/opt/trn_rl_repo")

import concourse.bass as bass  # noqa: E402
import concourse.tile as tile  # noqa: E402
from concourse import bacc, mybir  # noqa: E402
from concourse import bass_utils  # noqa: E402
from concourse._compat import with_exitstack  # noqa: E402

B, D, O, NCORES = 8192, 64, 256, 8
BSH = B // NCORES  # 1024 per-core batch shard
BT = 512
NBT = BSH // BT  # 2
NSC = 16  # s^2 chunks (2016 strict-upper pairs -> 15.75 chunks)
NCHUNK = NSC + 1  # + diag/linear chunk 0
F32 = mybir.dt.float32
F16 = mybir.dt.float16

# slot maps (compile-time constants)
_TRIU_D, _TRIU_F = np.triu_indices(D, k=1)  # 2016 pairs, row-major


def _row_segments():
    """Per off-diag row d: list of (chunk, part0, f0, seglen) covering its slots."""
    segs = []
    s = 0
    for d in range(D - 1):
        length = D - 1 - d
        f0 = d + 1
        while length > 0:
            c = 1 + s // 128
            p0 = s % 128
            seg = min(length, 128 - p0)
            segs.append((d, c, p0, f0, seg))
            s += seg
            f0 += seg
            length -= seg
    return segs


@with_exitstack
def _kernel(ctx: ExitStack, tc, outT, xT, betasT, indc, linW, negq3):
    nc = tc.nc

    cpool = ctx.enter_context(tc.tile_pool(name="const", bufs=1))
    dpool = ctx.enter_context(tc.tile_pool(name="dram", bufs=1, space="DRAM"))
    ge_pool = ctx.enter_context(tc.tile_pool(name="psum_ge", bufs=2, space="PSUM"))
    go_pool = ctx.enter_context(tc.tile_pool(name="psum_go", bufs=2, space="PSUM"))
    s_pool = ctx.enter_context(tc.tile_pool(name="psum_s", bufs=2, space="PSUM"))
    q_pool = ctx.enter_context(tc.tile_pool(name="psum_q", bufs=1, space="PSUM"))
    opool = ctx.enter_context(tc.tile_pool(name="outs", bufs=4))

    # ---- resident inputs (small ones first so PE work can start early) ----
    xb = cpool.tile([128, BSH], F16)  # [xT; xT]
    nc.sync.dma_start(xb[0:D, :], xT[:])
    nc.sync.dma_start(xb[D:128, :], xT[:])
    sb_indc = cpool.tile([128, 8 * 128], F16)
    nc.sync.dma_start(sb_indc[:], indc[:])
    w = cpool.tile([128, NCHUNK * 256], F16)  # [slot, (chunk, o')]
    nc.sync.dma_start(w[:, 0:256], linW[:])  # chunk 0 from host
    sb_nq3 = cpool.tile([128, 2], F32)
    nc.sync.dma_start(sb_nq3[:], negq3[:])
    sb_betasT = cpool.tile([128, 64 * 128], F16)  # row r: pairs tt%2==r
    for k in range(4):
        nc.sync.dma_start(
            sb_betasT[:, k * 2048 : (k + 1) * 2048],
            betasT[:, k * 2048 : (k + 1) * 2048],
        )

    # ---- g chunks ----
    g = [
        cpool.tile([128, BSH], F16, name=f"g{c}", uniquify=False)
        for c in range(NCHUNK)
    ]
    # chunk 0: [x^2 ; x]
    nc.sync.dma_start(g[0][D:128, :], xT[:])
    nc.vector.tensor_mul(g[0][0:D, :], xb[0:D, :], xb[0:D, :])

    # ---- interleaved: s-chunks (PE warmup) + Gram groups + copies ----
    p_sb = cpool.tile([D, D * 256], F16)  # [f, (f2, o')]

    def emit_s_chunk(c):
        b = (c - 1) // 2
        r0 = 0 if c % 2 == 1 else 64
        for bt in range(NBT):
            ps = s_pool.tile([128, BT], F32, name=f"s_{c}_{bt}", tag="s")
            nc.tensor.matmul(
                ps[:],
                sb_indc[r0 : r0 + 64, b * 128 : (b + 1) * 128],
                xb[r0 : r0 + 64, bt * BT : (bt + 1) * BT],
                start=True,
                stop=True,
            )
            if bt == 0:
                nc.scalar.activation(
                    g[c][:, bt * BT : (bt + 1) * BT],
                    ps[:],
                    mybir.ActivationFunctionType.Square,
                )
            else:
                tmp = opool.tile([128, BT], F16, name=f"st_{c}", tag="st")
                nc.vector.tensor_copy(tmp[:], ps[:])
                nc.gpsimd.tensor_mul(
                    g[c][:, bt * BT : (bt + 1) * BT], tmp[:], tmp[:]
                )

    def emit_gram_group(gidx):
        for r in range(2):  # tile-row: even/odd pairs
            pool = ge_pool if r == 0 else go_pool
            pp = pool.tile([128, 512], F32, name=f"pp_{gidx}_{r}", tag=f"pp{r}")
            for q in range(4):
                m = gidx * 4 + q
                blk = sb_betasT[r * 64 : r * 64 + 64, m * 128 : (m + 1) * 128]
                nc.tensor.matmul(
                    pp[:, q * 128 : (q + 1) * 128], blk, blk, start=True, stop=True
                )
            # copies: src partitions (j, f1), free (f2 of j2==j, q);
            # dst p_sb[f1, f2*256 + (j*128 + r*64 + 4*gidx + q)] -- 4-contig o'
            for j in range(2):
                src = pp[j * 64 : (j + 1) * 64, :].rearrange(
                    "p (q j2 f) -> p j2 f q", q=4, j2=2
                )[:, j, :, :]
                dstv = p_sb[:].rearrange(
                    "p (f j r gi q) -> p j r gi f q", j=2, r=2, gi=16, q=4
                )
                dst = dstv[:, j, r, gidx, :, :]
                if j == 0:
                    nc.scalar.activation(
                        dst, src, mybir.ActivationFunctionType.Copy
                    )
                else:
                    nc.vector.tensor_copy(dst, src)

    # s-chunks 1-3 first (PE warm-up while betasT streams in), then 1:1
    emit_s_chunk(1)
    emit_s_chunk(2)
    emit_s_chunk(3)
    nxt = 4
    for gidx in range(16):
        emit_gram_group(gidx)
        if nxt <= NSC:
            emit_s_chunk(nxt)
            nxt += 1

    # ---- W path: compact triangle row-writes -> one affine read ----
    p_dram = dpool.tile([2176, 256], F16)  # [tri-slot, o'] (2016 used)
    p_sb_v = p_sb[:].rearrange("p (f o) -> p f o", o=256)
    srow = 0
    for d in range(D - 1):
        ln = D - 1 - d
        nc.sync.dma_start(
            p_dram[srow : srow + ln, :], p_sb_v[d : d + 1, d + 1 : D, :]
        )
        srow += ln
    wsrc = p_dram[0:2048, :].rearrange("(c p) o -> p c o", p=128)
    nc.sync.dma_start(
        w[:, 256 : (1 + NSC) * 256].rearrange("p (c o) -> p c o", o=256), wsrc
    )

    # ---- mains: o-half serial; 2 PSUM banks (one per b-tile) ----
    for oh in range(2):
        pq = [
            q_pool.tile([128, BT], F32, name=f"pq_{oh}_{bt}", tag=f"pq{bt}")
            for bt in range(NBT)
        ]
        for c in range(NCHUNK):
            kc = 96 if c == NSC else 128
            for bt in range(NBT):
                nc.tensor.matmul(
                    pq[bt][:],
                    w[0:kc, c * 256 + oh * 128 : c * 256 + oh * 128 + 128],
                    g[c][0:kc, bt * BT : (bt + 1) * BT],
                    start=(c == 0),
                    stop=(c == NCHUNK - 1),
                )
        for bt in range(NBT):
            ob = opool.tile([128, BT], F16, name=f"ob_{oh}_{bt}", tag="ob")
            nc.scalar.activation(
                ob[:],
                pq[bt][:],
                mybir.ActivationFunctionType.Exp,
                scale=-1.0,
                bias=sb_nq3[:, oh : oh + 1],
            )
            nc.sync.dma_start(
                outT[oh * 128 : (oh + 1) * 128, bt * BT : (bt + 1) * BT], ob[:]
            )


_CACHE = {}


def _build():
    if "nc" in _CACHE:
        return _CACHE["nc"], _CACHE["aps"]
    nc = bacc.Bacc(
        "TRN2", target_bir_lowering=False, debug=False, num_devices=NCORES
    )
    xT = nc.dram_tensor("xT", [D, BSH], F16, kind="ExternalInput").ap()
    betasT = nc.dram_tensor("betasT", [128, 64 * 128], F16, kind="ExternalInput").ap()
    indc = nc.dram_tensor("indc", [128, 8 * 128], F16, kind="ExternalInput").ap()
    linW = nc.dram_tensor("linW", [128, 256], F16, kind="ExternalInput").ap()
    negq3 = nc.dram_tensor("negq3", [128, 2], F32, kind="ExternalInput").ap()
    outT = nc.dram_tensor("outT", [O, BSH], F16, kind="ExternalOutput").ap()
    with tile.TileContext(nc) as tc:
        _kernel(tc, outT, xT, betasT, indc, linW, negq3)
    nc.compile()
    _CACHE["nc"] = nc
    _CACHE["aps"] = (xT, betasT, indc, linW, negq3, outT)
    return nc, _CACHE["aps"]


def _operm():
    # o' = j*128 + r*64 + m,  j = o%2, tt = o//2, r = tt%2, m = tt//2
    o = np.arange(O)
    tt = o // 2
    return (o % 2) * 128 + (tt % 2) * 64 + tt // 2


def _host_prep(x, centers, betas):
    x = np.asarray(x, np.float32)
    betas = np.asarray(betas, np.float32)
    c = np.asarray(centers, np.float32).reshape(O, D)
    operm = _operm()

    # betasT stacked for row-tiled Gram: row-block r holds pairs tt with
    # tt%2==r; block m covers pair tt=2m+r -> o = (4m+2r, 4m+2r+1)
    bt_all = betas.transpose(2, 0, 1)  # [e, o, f]
    ev = np.stack([bt_all[:, 4 * m : 4 * m + 2, :] for m in range(64)], axis=1)
    od = np.stack([bt_all[:, 4 * m + 2 : 4 * m + 4, :] for m in range(64)], axis=1)
    betasT = np.concatenate(
        [ev.reshape(D, 64 * 128), od.reshape(D, 64 * 128)], axis=0
    ).astype(np.float16)

    # indc: two-hot columns for s-chunks; chunk c odd -> rows 0:64 of block
    # (c-1)//2, c even -> rows 64:128
    indc = np.zeros((128, 8 * 128), np.float32)
    for cc in range(1, NSC + 1):
        b = (cc - 1) // 2
        r0 = 0 if cc % 2 == 1 else 64
        lo = (cc - 1) * 128
        hi = min(lo + 128, 2016)
        for p in range(hi - lo):
            s = lo + p
            indc[r0 + _TRIU_D[s], b * 128 + p] = 1.0
            indc[r0 + _TRIU_F[s], b * 128 + p] = 1.0
    indc = indc.astype(np.float16)

    # host linear prep (~5M MACs)
    s1 = betas.sum(axis=1)  # [O, e]
    rowsum = np.einsum("ode,oe->od", betas, s1)  # [O, d]
    pdd = (betas ** 2).sum(axis=2)  # [O, d]
    diagW = 2.0 * pdd - rowsum  # [O, d]
    wvec = np.einsum("ofe,of->oe", betas, c)
    v = np.einsum("ode,oe->od", betas, wvec)
    q3 = (wvec ** 2).sum(axis=1)

    linW = np.zeros((128, 256), np.float32)
    linW[0:D, operm] = diagW.T
    linW[D:128, operm] = (-2.0 * v).T
    linW = linW.astype(np.float16)

    negq3 = np.zeros((128, 2), np.float32)
    for oh in range(2):
        inv = np.empty(O, np.int64)
        inv[operm] = np.arange(O)
        negq3[:, oh] = -q3[inv[oh * 128 : (oh + 1) * 128]]

    xT_shards = [
        np.ascontiguousarray(x[i * BSH : (i + 1) * BSH].T).astype(np.float16)
        for i in range(NCORES)
    ]
    return xT_shards, betasT, indc, linW, negq3


def _run(x, centers, betas, trace=False):
    nc, (xT, betasT_ap, indc_ap, linW_ap, negq3_ap, outT) = _build()
    xT_shards, betasT, indc, linW, negq3 = _host_prep(x, centers, betas)
    in_maps = [
        {
            xT.name: xT_shards[i],
            betasT_ap.name: betasT,
            indc_ap.name: indc,
            linW_ap.name: linW,
            negq3_ap.name: negq3,
        }
        for i in range(NCORES)
    ]
    res = bass_utils.run_bass_kernel_spmd(
        nc, in_maps, core_ids=list(range(NCORES)), trace=trace
    )
    operm = _operm()
    out = np.concatenate(
        [
            np.asarray(res.results[i][outT.name])[operm, :].T.astype(np.float32)
            for i in range(NCORES)
        ],
        axis=0,
    )
    return out, res


def kernel(x, centers, betas):
    out, _ = _run(x, centers, betas, trace=False)
    return out


# revision 14
# speedup vs baseline: 2.0662x; 1.5996x over previous
"""Trainium2 Bass kernel for nn_Cov_EBFLayer (v3: padded-triangle s^2 features).

Math: out[b,o] = exp(-quad[o,b]),
  quad[o,b] = diff^T P_o diff,  diff = c_o - x_b,  P_o = B_o B_o^T
            = sum_{d<f} P[d,f] (x_d+x_f)^2            <- s^2 features (2016)
            + sum_d (2 P[d,d] - rowsum_d) x_d^2       <- diag features (64)
            - 2 v_o . x + q3_o                        <- linear (64) + exp bias
  The (x_d+x_f)^2 expansion absorbs the symmetric cross terms; its spurious
  x^2 terms fold into the diagonal coefficients (host, ~5M MACs).

Feature slots live in a GROUP-PADDED triangle space so the whole W reshape
is affine: rows d in [8k, 8k+8) are padded to uniform width w_k = 63-8k
(f in [8k+1, 64)); slots where f <= d are junk (indicator column = 0 so
g = 0 there, junk W x 0 = 0). 2240 padded slots -> 18 s-chunks of 128.

Per core (batch-sharded 8 x 1024):
  - Gram first on PE: 128 pair-Grams row-tiled (even pairs K-rows 0:64, odd
    64:128), PSUM->SBUF strided copies (ACT j=0 / DVE j=1) into
    p_sb[f1, (f2, o')], o' = j*128 + r*64 + m (4-contiguous per copy).
  - W path: 8 affine group-writes p_sb->p_dram[2304,256], 4 chunked affine
    reads -> W[slot, (chunk, o')].  Chunk 0 of W comes from host (diag+lin).
  - s-chunks: s = indc-matmul (two-hot, K=64, base-partition alternating),
    g = Square(s): ACT direct (bt0) or DVE copy+mul (bt1).
  - mains: per o-half (serial, 2 PSUM banks): 20 accumulating matmuls per
    b-tile; epilogue EXP(scale=-1, bias=-q3[o']) -> fp16 out.
"""

import sys
from contextlib import ExitStack

import numpy as np

sys.path.insert(0, "/opt/trn_rl_repo")

import concourse.bass as bass  # noqa: E402
import concourse.tile as tile  # noqa: E402
from concourse import bacc, mybir  # noqa: E402
from concourse import bass_utils  # noqa: E402
from concourse._compat import with_exitstack  # noqa: E402

B, D, O, NCORES = 8192, 64, 256, 8
BSH = B // NCORES  # 1024 per-core batch shard
BT = 512
NBT = BSH // BT  # 2
NSC = 18  # s^2 chunks over the 2240-slot padded triangle
NCHUNK = NSC + 1  # + diag/linear chunk 0
NPAD = 2240
F32 = mybir.dt.float32
F16 = mybir.dt.float16

_GBASE = []  # padded-row base per group of 8 d-rows
_GW = []
_b = 0
for _k in range(8):
    _w = 63 - 8 * _k
    _GBASE.append(_b)
    _GW.append(_w)
    _b += 8 * _w


def _slot_df(sl):
    """padded slot -> (d, f) or None if junk."""
    for k in range(8):
        if sl < _GBASE[k] + 8 * _GW[k]:
            off = sl - _GBASE[k]
            d = 8 * k + off // _GW[k]
            f = 8 * k + 1 + off % _GW[k]
            return (d, f) if f > d else None
    return None


@with_exitstack
def _kernel(ctx: ExitStack, tc, outT, xT, betasT, indc, linW, negq3):
    nc = tc.nc

    cpool = ctx.enter_context(tc.tile_pool(name="const", bufs=1))
    dpool = ctx.enter_context(tc.tile_pool(name="dram", bufs=1, space="DRAM"))
    ge_pool = ctx.enter_context(tc.tile_pool(name="psum_ge", bufs=2, space="PSUM"))
    go_pool = ctx.enter_context(tc.tile_pool(name="psum_go", bufs=2, space="PSUM"))
    s_pool = ctx.enter_context(tc.tile_pool(name="psum_s", bufs=2, space="PSUM"))
    q_pool = ctx.enter_context(tc.tile_pool(name="psum_q", bufs=1, space="PSUM"))
    opool = ctx.enter_context(tc.tile_pool(name="outs", bufs=4))

    # ---- inputs (small first; betasT chunked so Gram starts early) ----
    xb = cpool.tile([128, BSH], F16)  # [xT; xT]
    nc.sync.dma_start(xb[0:D, :], xT[:])
    nc.sync.dma_start(xb[D:128, :], xT[:])
    sb_indc = cpool.tile([128, 9 * 128], F16)
    nc.sync.dma_start(sb_indc[:], indc[:])
    w = cpool.tile([128, NCHUNK * 256], F16)  # [slot, (chunk, o')]
    nc.sync.dma_start(w[:, 0:256], linW[:])  # chunk 0 from host
    sb_nq3 = cpool.tile([128, 2], F32)
    nc.sync.dma_start(sb_nq3[:], negq3[:])
    sb_betasT = cpool.tile([128, 64 * 128], F16)  # row r: pairs tt%2==r
    for k in range(4):
        nc.sync.dma_start(
            sb_betasT[:, k * 2048 : (k + 1) * 2048],
            betasT[:, k * 2048 : (k + 1) * 2048],
        )

    # ---- g chunks ----
    g = [
        cpool.tile([128, BSH], F16, name=f"g{c}", uniquify=False)
        for c in range(NCHUNK)
    ]
    # chunk 0: [x^2 ; x]
    nc.sync.dma_start(g[0][D:128, :], xT[:])
    nc.vector.tensor_mul(g[0][0:D, :], xb[0:D, :], xb[0:D, :])

    def emit_s_chunk(c):
        b = (c - 1) // 2
        r0 = 0 if c % 2 == 1 else 64
        for bt in range(NBT):
            ps = s_pool.tile([128, BT], F32, name=f"s_{c}_{bt}", tag="s")
            nc.tensor.matmul(
                ps[:],
                sb_indc[r0 : r0 + 64, b * 128 : (b + 1) * 128],
                xb[r0 : r0 + 64, bt * BT : (bt + 1) * BT],
                start=True,
                stop=True,
            )
            if bt == 0:
                nc.scalar.activation(
                    g[c][:, bt * BT : (bt + 1) * BT],
                    ps[:],
                    mybir.ActivationFunctionType.Square,
                )
            else:
                tmp = opool.tile([128, BT], F16, name=f"st_{c}", tag="st")
                nc.vector.tensor_copy(tmp[:], ps[:])
                nc.vector.tensor_mul(
                    g[c][:, bt * BT : (bt + 1) * BT], tmp[:], tmp[:]
                )

    p_sb = cpool.tile([D, D * 256], F16)  # [f1, (f2, o')]

    def emit_gram_group(gidx):
        for r in range(2):  # tile-row: even/odd pairs
            pool = ge_pool if r == 0 else go_pool
            pp = pool.tile([128, 512], F32, name=f"pp_{gidx}_{r}", tag=f"pp{r}")
            for q in range(4):
                m = gidx * 4 + q
                blk = sb_betasT[r * 64 : r * 64 + 64, m * 128 : (m + 1) * 128]
                nc.tensor.matmul(
                    pp[:, q * 128 : (q + 1) * 128], blk, blk, start=True, stop=True
                )
            # copies: src partitions (j, f1), free (f2 of j2==j, q);
            # dst p_sb[f1, f2*256 + (j*128 + r*64 + 4*gidx + q)]
            for j in range(2):
                src = pp[j * 64 : (j + 1) * 64, :].rearrange(
                    "p (q j2 f) -> p j2 f q", q=4, j2=2
                )[:, j, :, :]
                dstv = p_sb[:].rearrange(
                    "p (f j r gi q) -> p j r gi f q", j=2, r=2, gi=16, q=4
                )
                dst = dstv[:, j, r, gidx, :, :]
                if j == 0:
                    nc.scalar.activation(
                        dst, src, mybir.ActivationFunctionType.Copy
                    )
                else:
                    nc.vector.tensor_copy(dst, src)

    # ---- emission: 2 s-chunks as cold warm-up, all Gram, rest of s ----
    emit_s_chunk(1)
    emit_s_chunk(2)
    for gidx in range(16):
        emit_gram_group(gidx)
    for c in range(3, NSC + 1):
        emit_s_chunk(c)

    # ---- W path: 8 affine padded group-writes -> 4 affine chunked reads ----
    p_dram = dpool.tile([2304, 256], F16)  # [padded-slot, o']
    p_sb_v = p_sb[:].rearrange("p (f o) -> p f o", o=256)
    for k in range(8):
        nc.sync.dma_start(
            p_dram[_GBASE[k] : _GBASE[k] + 8 * _GW[k], :],
            p_sb_v[8 * k : 8 * k + 8, 8 * k + 1 : D, :],
        )
    wv = w[:, 256:].rearrange("p (c o) -> p c o", o=256)
    rd_bounds = [0, 4, 8, 13, 18]  # chunk-granular read splits
    for i in range(4):
        c0, c1 = rd_bounds[i], rd_bounds[i + 1]
        nc.sync.dma_start(
            wv[:, c0:c1, :],
            p_dram[c0 * 128 : c1 * 128, :].rearrange("(c p) o -> p c o", p=128),
        )

    # ---- mains: o-half serial; 2 PSUM banks (one per b-tile) ----
    for oh in range(2):
        pq = [
            q_pool.tile([128, BT], F32, name=f"pq_{oh}_{bt}", tag=f"pq{bt}")
            for bt in range(NBT)
        ]
        for c in range(NCHUNK):
            kc = 64 if c == NSC else 128
            for bt in range(NBT):
                nc.tensor.matmul(
                    pq[bt][:],
                    w[0:kc, c * 256 + oh * 128 : c * 256 + oh * 128 + 128],
                    g[c][0:kc, bt * BT : (bt + 1) * BT],
                    start=(c == 0),
                    stop=(c == NCHUNK - 1),
                )
        for bt in range(NBT):
            ob = opool.tile([128, BT], F16, name=f"ob_{oh}_{bt}", tag="ob")
            nc.scalar.activation(
                ob[:],
                pq[bt][:],
                mybir.ActivationFunctionType.Exp,
                scale=-1.0,
                bias=sb_nq3[:, oh : oh + 1],
            )
            nc.sync.dma_start(
                outT[oh * 128 : (oh + 1) * 128, bt * BT : (bt + 1) * BT], ob[:]
            )


_CACHE = {}


def _build():
    if "nc" in _CACHE:
        return _CACHE["nc"], _CACHE["aps"]
    nc = bacc.Bacc(
        "TRN2", target_bir_lowering=False, debug=False, num_devices=NCORES
    )
    xT = nc.dram_tensor("xT", [D, BSH], F16, kind="ExternalInput").ap()
    betasT = nc.dram_tensor("betasT", [128, 64 * 128], F16, kind="ExternalInput").ap()
    indc = nc.dram_tensor("indc", [128, 9 * 128], F16, kind="ExternalInput").ap()
    linW = nc.dram_tensor("linW", [128, 256], F16, kind="ExternalInput").ap()
    negq3 = nc.dram_tensor("negq3", [128, 2], F32, kind="ExternalInput").ap()
    outT = nc.dram_tensor("outT", [O, BSH], F16, kind="ExternalOutput").ap()
    with tile.TileContext(nc) as tc:
        _kernel(tc, outT, xT, betasT, indc, linW, negq3)
    nc.compile()
    _CACHE["nc"] = nc
    _CACHE["aps"] = (xT, betasT, indc, linW, negq3, outT)
    return nc, _CACHE["aps"]


def _operm():
    # o' = j*128 + r*64 + m,  j = o%2, tt = o//2, r = tt%2, m = tt//2
    o = np.arange(O)
    tt = o // 2
    return (o % 2) * 128 + (tt % 2) * 64 + tt // 2


def _host_prep(x, centers, betas):
    x = np.asarray(x, np.float32)
    betas = np.asarray(betas, np.float32)
    cen = np.asarray(centers, np.float32).reshape(O, D)
    operm = _operm()

    # betasT stacked for row-tiled Gram: row-block r holds pairs tt%2==r;
    # block m of row r covers pair tt=2m+r -> o = (4m+2r, 4m+2r+1)
    bt_all = betas.transpose(2, 0, 1)  # [e, o, f]
    ev = np.stack([bt_all[:, 4 * m : 4 * m + 2, :] for m in range(64)], axis=1)
    od = np.stack([bt_all[:, 4 * m + 2 : 4 * m + 4, :] for m in range(64)], axis=1)
    betasT = np.concatenate(
        [ev.reshape(D, 64 * 128), od.reshape(D, 64 * 128)], axis=0
    ).astype(np.float16)

    # indc: two-hot columns for real padded slots, zero for junk
    indc = np.zeros((128, 9 * 128), np.float32)
    for cc in range(1, NSC + 1):
        blk = (cc - 1) // 2
        r0 = 0 if cc % 2 == 1 else 64
        for p in range(128):
            sl = (cc - 1) * 128 + p
            if sl >= NPAD:
                break
            df = _slot_df(sl)
            if df is None:
                continue
            d, f = df
            indc[r0 + d, blk * 128 + p] = 1.0
            indc[r0 + f, blk * 128 + p] = 1.0
    indc = indc.astype(np.float16)

    # host linear prep (~5M MACs)
    s1 = betas.sum(axis=1)  # [O, e]
    rowsum = np.einsum("ode,oe->od", betas, s1)
    pdd = (betas ** 2).sum(axis=2)
    diagW = 2.0 * pdd - rowsum
    wvec = np.einsum("ofe,of->oe", betas, cen)
    v = np.einsum("ode,oe->od", betas, wvec)
    q3 = (wvec ** 2).sum(axis=1)

    linW = np.zeros((128, 256), np.float32)
    linW[0:D, operm] = diagW.T
    linW[D:128, operm] = (-2.0 * v).T
    linW = linW.astype(np.float16)

    negq3 = np.zeros((128, 2), np.float32)
    inv = np.empty(O, np.int64)
    inv[operm] = np.arange(O)
    for oh in range(2):
        negq3[:, oh] = -q3[inv[oh * 128 : (oh + 1) * 128]]

    xT_shards = [
        np.ascontiguousarray(x[i * BSH : (i + 1) * BSH].T).astype(np.float16)
        for i in range(NCORES)
    ]
    return xT_shards, betasT, indc, linW, negq3


def _run(x, centers, betas, trace=False):
    nc, (xT, betasT_ap, indc_ap, linW_ap, negq3_ap, outT) = _build()
    xT_shards, betasT, indc, linW, negq3 = _host_prep(x, centers, betas)
    in_maps = [
        {
            xT.name: xT_shards[i],
            betasT_ap.name: betasT,
            indc_ap.name: indc,
            linW_ap.name: linW,
            negq3_ap.name: negq3,
        }
        for i in range(NCORES)
    ]
    res = bass_utils.run_bass_kernel_spmd(
        nc, in_maps, core_ids=list(range(NCORES)), trace=trace
    )
    operm = _operm()
    out = np.concatenate(
        [
            np.asarray(res.results[i][outT.name])[operm, :].T.astype(np.float32)
            for i in range(NCORES)
        ],
        axis=0,
    )
    return out, res


def kernel(x, centers, betas):
    out, _ = _run(x, centers, betas, trace=False)
    return out
